# revision 1
# baseline (speedup 1.0000x reference)
"""Trainium2 Bass kernel for causal cosine-sim attention block (8 cores).

Reference computation (per problem):
  x [2, 2048, 1024] fp32
  xn = LayerNorm(x) * ln_w + ln_b
  qkv = xn @ W_qkv  -> q, k, v   (16 heads x 64)
  q, k l2-normalized per head-dim; sim = (q.k) * 8, causal mask, softmax
  o = attn @ v ; out = o @ W_out   [2, 2048, 1024] fp32

Sharding (8 cores):
  - LayerNorm: replicated (each core normalizes all 4096 tokens; x fed bf16).
  - QKV projection + attention: head-parallel. Core c owns heads {2c, 2c+1}
    for both batches (column-sharded W_qkv).
  - Out projection: token-parallel. Two AllToAlls (one per batch) exchange
    the head-sharded attention outputs o^T for token shards; the batch-0
    exchange overlaps batch-1 attention. Core c computes output rows
    [256c, 256(c+1)) of each batch with full W_out.

Engine balance notes (v5):
  - LayerNorm is folded into the QKV matmul: q/k need only a rank-1 mean
    correction (x@W - mu (x) colsum(W), done as a K=1 matmul on the PE;
    the rstd factor cancels under l2 normalization), and v gets a
    per-token rstd scale on DVE. No normalized-x ACT pass at all.
  - ACT keeps only: attention exp, rstd sqrt, one batched Sqrt per batch.
    (NOTE: nc.vector.tensor_tensor_reduce hangs real HW -- use
    tensor_tensor + reduce_sum.)
  - Attention computed in S^T orientation: S^T[k, q] tiles; exp on ACT
    (scale=8 folded in) -> E bf16; PV matmul uses lhsT = [V | 1] so
    partition 64 of the PSUM output accumulates softmax denominators.
  - Per-batch staging buffers (qkv fp32, norms) so batch b+1's QKV matmuls
    overlap batch b's normalize/transpose tail on other engines.
  - Causality at 128-col block granularity: fully-hidden blocks skipped,
    diagonal blocks masked with gpsimd.affine_select on the E tile.
"""

import numpy as np

import concourse.bass as bass
import concourse.mybir as mybir
import concourse.tile as tile
from concourse import bacc
from concourse.bass import ts, ds

F32 = mybir.dt.float32
BF16 = mybir.dt.bfloat16

NCORES = 8
DIM = 1024
HEADS = 16
DHEAD = 64
INNER = HEADS * DHEAD          # 1024
B = 2
N = 2048
NTOK = B * N                   # 4096
TOK_HALF = N // NCORES         # 256 tokens per core per batch
TOK_SLICE = B * TOK_HALF       # 512 output rows per core
HLOC = HEADS // NCORES         # 2 heads per core
QKV_COLS = 3 * HLOC * DHEAD    # 384
EPS = 1e-5
SCALE = 8.0
P = 128
KT_PER_B = N // P              # 16 k-tiles / token tiles per batch
QB_PER_B = N // 512            # 4 q-blocks (512) per batch
AluOp = mybir.AluOpType
Act = mybir.ActivationFunctionType


def build_kernel():
    nc = bacc.Bacc("TRN2", target_bir_lowering=False, debug=False,
                   num_devices=NCORES)

    x_in = nc.dram_tensor("x_t", [NTOK, DIM], BF16, kind="ExternalInput")
    w_qkv = nc.dram_tensor("w_qkv", [DIM, QKV_COLS + 1], BF16,
                           kind="ExternalInput")
    w_out = nc.dram_tensor("w_out", [INNER, DIM], BF16, kind="ExternalInput")
    neg_cs = nc.dram_tensor("neg_cs", [1, QKV_COLS], F32,
                            kind="ExternalInput")
    y_out = nc.dram_tensor("y_out", [B, TOK_HALF, DIM], F32,
                           kind="ExternalOutput")

    with tile.TileContext(nc) as tc:
        _body(nc, tc, x_in, w_qkv, w_out, neg_cs, y_out)
    nc.compile()
    return nc


def _body(nc, tc, x_in, w_qkv, w_out, neg_cs, y_out):
    import contextlib
    ctx = contextlib.ExitStack()
    with ctx:
        persist = ctx.enter_context(tc.tile_pool(name="persist", bufs=1))
        ln_pool = ctx.enter_context(tc.tile_pool(name="ln", bufs=4))
        small = ctx.enter_context(tc.tile_pool(name="small", bufs=4))
        xnt_pool = ctx.enter_context(tc.tile_pool(name="xnt", bufs=4))
        qkv_ps_pool = ctx.enter_context(
            tc.tile_pool(name="qkvps", bufs=2, space="PSUM"))
        out_ps_pool = ctx.enter_context(
            tc.tile_pool(name="outps", bufs=2, space="PSUM"))
        st_ps_pool = ctx.enter_context(
            tc.tile_pool(name="stps", bufs=2, space="PSUM"))
        o_ps_pool = ctx.enter_context(
            tc.tile_pool(name="ops", bufs=1, space="PSUM"))
        e_pool = ctx.enter_context(tc.tile_pool(name="epool", bufs=6))
        norm_pool = ctx.enter_context(tc.tile_pool(name="norm", bufs=2))
        out_pool = ctx.enter_context(tc.tile_pool(name="outp", bufs=3))
        dram = ctx.enter_context(tc.tile_pool(name="dram", bufs=1,
                                              space="DRAM"))

        # ---- persistent SBUF buffers (per-partition bytes noted) ----
        w_qkv_sb = persist.tile([P, DIM // P, QKV_COLS + 1], BF16)
        qkT = persist.tile([P, 2, B, N], BF16)                   # 16 KB
        v_sb = persist.tile([P, B, KT_PER_B, HLOC, DHEAD + 1], BF16)  # 8.3
        # unnormalized attention out + denoms (one batch at a time)
        oU = persist.tile([DHEAD + 1, QB_PER_B, HLOC, 512], F32)  # 16 KB
        oT = persist.tile([P, B, N], BF16)                       # 8 KB
        qkvf = persist.tile([P, B, KT_PER_B, 256], F32)          # 32 KB
        ssq_all = persist.tile([P, B, KT_PER_B, 4], F32)
        rcp_all = persist.tile([P, B, KT_PER_B, 4], F32)
        oT_all = persist.tile([P, INNER // P, B, TOK_HALF], BF16)  # 8 KB
        w_out_sb = persist.tile([P, INNER // P, DIM], BF16)      # 16 KB

        nc.sync.dma_start(
            w_qkv_sb[:], w_qkv.ap().rearrange("(o p) c -> p o c", p=P))
        nc.sync.dma_start(
            w_out_sb[:], w_out.ap().rearrange("(o p) c -> p o c", p=P))
        nc.vector.memset(v_sb[:, :, :, :, DHEAD], 1.0)
        eps_t = persist.tile([P, 1], F32)
        nc.vector.memset(eps_t[:], EPS)
        negcs_sb = persist.tile([1, QKV_COLS], F32)
        nc.sync.dma_start(negcs_sb[:], neg_cs.ap())
        negcs_bc = persist.tile([P, QKV_COLS], F32)
        nc.gpsimd.partition_broadcast(negcs_bc[:], negcs_sb[:])

        cc_in = []
        cc_out = []
        for bi in range(B):
            cci = dram.tile([NCORES, P, TOK_HALF], BF16, name=f"cci{bi}")
            cco = dram.tile([NCORES, P, TOK_HALF], BF16, name=f"cco{bi}")
            cc_in.append(cci)
            cc_out.append(cco)

        # ============ Stage A+B per batch: LN -> xn^T -> QKV =============
        for bi in range(B):
            for ti in range(KT_PER_B):
                i = bi * KT_PER_B + ti
                xt = ln_pool.tile([P, DIM], BF16, tag="xt")
                nc.sync.dma_start(xt[:], x_in.ap()[ts(i, P), :])

                # transpose raw x immediately (no LN dependency)
                xnt = xnt_pool.tile([P, DIM // P, P], BF16, tag="xnt")
                nc.scalar.dma_start_transpose(xnt[:], xt[:])

                # LN stats (only rstd is needed, for the V scale)
                stats = small.tile([P, 2, 6], F32, tag="stats")
                nc.vector.bn_stats(stats[:, 0, :], xt[:, 0:512])
                nc.vector.bn_stats(stats[:, 1, :], xt[:, 512:1024])
                mv = small.tile([P, 2], F32, tag="mv")
                nc.vector.bn_aggr(mv[:], stats[:])
                rstd = small.tile([P, 1], F32, tag="rstd")
                nc.scalar.activation(rstd[:], mv[:, 1:2], Act.Sqrt,
                                     bias=eps_t[:])
                nc.vector.reciprocal(rstd[:], rstd[:])

                # QKV matmul on raw x^T; W's 385th column is 1/1024 so the
                # psum's last column accumulates the per-token mean for free
                qkv_ps = qkv_ps_pool.tile([P, QKV_COLS + 1], F32,
                                          tag="qkvps")
                for o in range(DIM // P):
                    nc.tensor.matmul(qkv_ps[:], lhsT=xnt[:, o, :],
                                     rhs=w_qkv_sb[:, o, :],
                                     start=(o == 0), stop=(o == DIM // P - 1))
                # rank-1 mean correction tile: corr = mu (x) (-colsum(W))
                mu_sb = small.tile([P, 1], F32, tag="mu_sb")
                nc.vector.tensor_copy(mu_sb[:],
                                      qkv_ps[:, QKV_COLS:QKV_COLS + 1])
                corr = small.tile([P, QKV_COLS], F32, tag="corr")
                nc.vector.tensor_scalar_mul(corr[:], negcs_bc[:], mu_sb[:])

                # v = rstd * (v' + corr_v) (bf16), ones column preset
                vtmp = small.tile([P, 2 * DHEAD], F32, tag="vtmp")
                nc.vector.tensor_tensor(vtmp[:], qkv_ps[:, 256:QKV_COLS],
                                        corr[:, 256:QKV_COLS], AluOp.add)
                nc.vector.tensor_scalar_mul(
                    v_sb[:, bi, ti, :, 0:DHEAD],
                    vtmp[:].rearrange("p (h d) -> p h d", d=DHEAD),
                    rstd[:])

                # corrected q,k to fp32 staging; squared norms
                nc.vector.tensor_tensor(qkvf[:, bi, ti, :],
                                        qkv_ps[:, 0:256], corr[:, 0:256],
                                        AluOp.add)
                sq = small.tile([P, 4 * DHEAD], F32, tag="sq")
                nc.vector.tensor_tensor(sq[:], qkvf[:, bi, ti, :],
                                        qkvf[:, bi, ti, :], AluOp.mult)
                nc.vector.reduce_sum(
                    ssq_all[:, bi, ti, :],
                    sq[:].rearrange("p (j d) -> p j d", d=DHEAD),
                    axis=mybir.AxisListType.X)

            # batched rsqrt of this batch's q/k norms: one ACT Sqrt
            nc.scalar.activation(
                rcp_all[:, bi].rearrange("p t j -> p (t j)"),
                ssq_all[:, bi].rearrange("p t j -> p (t j)"), Act.Sqrt)
            nc.vector.tensor_scalar_max(
                rcp_all[:, bi].rearrange("p t j -> p (t j)"),
                rcp_all[:, bi].rearrange("p t j -> p (t j)"), 1e-12)
            nc.vector.reciprocal(
                rcp_all[:, bi].rearrange("p t j -> p (t j)"),
                rcp_all[:, bi].rearrange("p t j -> p (t j)"))

            for ti in range(KT_PER_B):
                qkn = ln_pool.tile([P, 2 * P], BF16, tag="qkn")
                for j in range(4):
                    nc.vector.tensor_scalar_mul(
                        qkn[:, ts(j, DHEAD)], qkvf[:, bi, ti, ts(j, DHEAD)],
                        rcp_all[:, bi, ti, j:j + 1])
                nc.sync.dma_start_transpose(
                    qkT[:, :, bi, ts(ti, P)], qkn[:])

        # ========== Stage C: attention (+ per-batch normalize/A2A) =======
        for bi in range(B):
            for qb in range(QB_PER_B):
                o_ps = []
                for hh in range(HLOC):
                    o_ps_h = o_ps_pool.tile([1 + DHEAD, 512], F32,
                                            tag=f"ops{hh}", name=f"ops{hh}")
                    o_ps.append(o_ps_h)
                nkt = 4 * (qb + 1)
                for kt in range(nkt):
                    d = kt - 4 * qb  # >= 0 : diagonal block group
                    c0 = max(d, 0) * P
                    for hh in range(HLOC):
                        hsl = slice(hh * DHEAD, (hh + 1) * DHEAD)
                        st_ps = st_ps_pool.tile([P, 512], F32, tag="stps")
                        nc.tensor.matmul(
                            st_ps[:], lhsT=qkT[hsl, 1, bi, ts(kt, P)],
                            rhs=qkT[hsl, 0, bi, ds(qb * 512, 512)],
                            start=True, stop=True,
                            tile_position=(hh * DHEAD, 0))
                        e_t = e_pool.tile([P, 512], BF16, tag="et")
                        nc.scalar.activation(e_t[:, c0:512],
                                             st_ps[:, c0:512],
                                             Act.Exp, scale=SCALE)
                        if d >= 0:
                            # zero where q_local < k_local on the diag block
                            nc.gpsimd.affine_select(
                                out=e_t[:, c0:c0 + P],
                                in_=e_t[:, c0:c0 + P],
                                pattern=[[1, P]],
                                compare_op=AluOp.is_ge,
                                fill=0.0,
                                base=0,
                                channel_multiplier=-1)
                        nc.tensor.matmul(
                            o_ps[hh][:, c0:512],
                            lhsT=v_sb[:, bi, kt, hh, :],
                            rhs=e_t[:, c0:512],
                            start=(kt == 0), stop=(kt == nkt - 1))

                # evac unnormalized output + denominators (fp32)
                for hh in range(HLOC):
                    nc.vector.tensor_copy(oU[:, qb, hh, :], o_ps[hh][:])

            # ---- batched softmax normalization for this batch ----
            pack = norm_pool.tile([QB_PER_B * HLOC, 512], F32, tag="pack")
            for qb in range(QB_PER_B):
                for hh in range(HLOC):
                    r = qb * HLOC + hh
                    nc.sync.dma_start(pack[r:r + 1, :],
                                      oU[DHEAD:DHEAD + 1, qb, hh, :])
            nc.vector.reciprocal(pack[:], pack[:])
            for qb in range(QB_PER_B):
                for hh in range(HLOC):
                    r = qb * HLOC + hh
                    row0 = norm_pool.tile([1, 512], F32, tag="row0")
                    nc.sync.dma_start(row0[:], pack[r:r + 1, :])
                    bc = norm_pool.tile([DHEAD, 512], F32, tag="bc")
                    nc.gpsimd.partition_broadcast(bc[:], row0[:])
                    nc.vector.tensor_tensor(
                        oT[hh * DHEAD:(hh + 1) * DHEAD, bi,
                           ds(qb * 512, 512)],
                        oU[0:DHEAD, qb, hh, :], bc[:],
                        AluOp.mult)

            # ---- AllToAll for this batch (overlaps next batch's work) ----
            nc.sync.dma_start(
                cc_in[bi][:].rearrange("s p f -> p s f"),
                oT[:, bi, :].rearrange("p (s f) -> p s f", f=TOK_HALF))
            nc.gpsimd.collective_compute(
                "AllToAll", AluOp.bypass,
                replica_groups=[list(range(NCORES))],
                ins=[cc_in[bi].opt()], outs=[cc_out[bi].opt()])
            nc.sync.dma_start(oT_all[:, :, bi, :],
                              cc_out[bi][:].rearrange("s p f -> p s f"))

            # ---- out projection for this batch's token slice ----
            for tt in range(TOK_HALF // P):
                for half in range(2):
                    out_ps = out_ps_pool.tile([P, 512], F32, tag="outps")
                    for o in range(INNER // P):
                        nc.tensor.matmul(
                            out_ps[:], lhsT=oT_all[:, o, bi, ts(tt, P)],
                            rhs=w_out_sb[:, o, ds(half * 512, 512)],
                            start=(o == 0), stop=(o == INNER // P - 1))
                    ot = out_pool.tile([P, 512], F32, tag="ot")
                    nc.vector.tensor_copy(ot[:], out_ps[:])
                    nc.sync.dma_start(
                        y_out.ap()[bi, ts(tt, P), ds(half * 512, 512)],
                        ot[:])





# ----------------------------------------------------------------------
# Host side
# ----------------------------------------------------------------------

def make_in_maps(x, ln_w, ln_b, W_qkv, W_out):
    """Build the per-core input maps (host-side sharding/marshaling)."""
    import ml_dtypes
    x = np.asarray(x, dtype=np.float32)
    ln_w = np.asarray(ln_w, dtype=np.float32)
    ln_b = np.asarray(ln_b, dtype=np.float32)
    W_qkv = np.asarray(W_qkv, dtype=np.float32)
    W_out = np.asarray(W_out, dtype=np.float32)

    assert np.allclose(ln_b, 0.0), \
        "kernel folds ln_b@W into a bias; nonzero ln_b not wired up"

    x_t = np.ascontiguousarray(
        x.reshape(NTOK, DIM)).astype(ml_dtypes.bfloat16)
    w_eff = (ln_w[:, None] * W_qkv)  # [1024, 3072]
    q_w = w_eff[:, 0 * INNER:1 * INNER]
    k_w = w_eff[:, 1 * INNER:2 * INNER]
    v_w = w_eff[:, 2 * INNER:3 * INNER]
    w_out_bf = W_out.astype(ml_dtypes.bfloat16)

    in_maps = []
    for c in range(NCORES):
        h0, h1 = 2 * c, 2 * c + 2
        wq = q_w[:, h0 * DHEAD:h1 * DHEAD]
        wk = k_w[:, h0 * DHEAD:h1 * DHEAD]
        wv = v_w[:, h0 * DHEAD:h1 * DHEAD]
        mu_col = np.full((DIM, 1), 1.0 / DIM, dtype=np.float32)
        w_c_f32 = np.concatenate([wq, wk, wv, mu_col], axis=1)
        w_c = w_c_f32.astype(ml_dtypes.bfloat16)
        neg_cs = np.ascontiguousarray(
            -w_c.astype(np.float32)[:, 0:QKV_COLS]
            .sum(axis=0, keepdims=True), dtype=np.float32)
        in_maps.append({
            "x_t": x_t,
            "w_qkv": np.ascontiguousarray(w_c),
            "w_out": w_out_bf,
            "neg_cs": np.ascontiguousarray(neg_cs),
        })
    return in_maps


def gather_output(results):
    """results: list of per-core {name: array} -> full [2, 2048, 1024]."""
    full = np.empty((B, N, DIM), dtype=np.float32)
    for c in range(NCORES):
        part = results[c]["y_out"]  # [B, TOK_HALF, DIM]
        full[:, c * TOK_HALF:(c + 1) * TOK_HALF, :] = part
    return full


_NC_CACHE = None


def kernel(x, ln_w, ln_b, W_qkv, W_out):
    global _NC_CACHE
    from concourse.bass_utils import run_bass_kernel_spmd
    if _NC_CACHE is None:
        _NC_CACHE = build_kernel()
    in_maps = make_in_maps(x, ln_w, ln_b, W_qkv, W_out)
    res = run_bass_kernel_spmd(_NC_CACHE, in_maps,
                               core_ids=list(range(NCORES)))
    return gather_output(res.results)

